# revision 35
# baseline (speedup 1.0000x reference)
"""Two-layer GAT on 8 Trainium2 NeuronCores (Bass/Tile SPMD kernel).

Sharding: nodes are range-partitioned across the 8 cores; each core owns the
edges whose *destination* falls in its partition (segment softmax is per-dst,
so softmax/aggregation is fully core-local). Each layer's node-feature table
(h | a_src per row, bf16) is computed for the core's OWN nodes only and
AllGathered so every core can gather arbitrary source rows. a_dst tables stay
core-local.

Edge layout (per 128-dst block):
  * main pass: slot-aligned — edge for dst-slot p sits in partition row p,
    up to K columns. The per-edge scatter-add then reduces to PSUM
    accumulation of identity-lhsT matmuls (no per-edge one-hot needed), and
    a_dst[dst] is a stride-0 broadcast of a direct [128, H] load (no
    per-edge a_dst gather).
  * overflow pass: edges beyond K per dst go to densely packed columns and
    use the classic one-hot (slot == iota) matmul scatter plus a small
    per-edge a_dst gather.
  * padded main slots point at a dedicated PAD table row whose a_src is
    -1e38, so exp(lrelu(alpha)) == 0 and they contribute nothing.

Per-edge softmax uses the shift-free form out_d = sum ex*h[src] / sum ex
(alpha is O(1), exp cannot overflow), and exp(leaky_relu(x)) is
max(exp(x), exp(0.2 x)) so only the Exp activation table is ever loaded.
All matmuls and gathered tables are bf16 (PE: 1 cycle/row vs 4 for fp32).
Indirect gathers are batched over groups of R=4 blocks to amortize the
~1us fixed descriptor-generation cost per indirect DMA.
"""

import sys

sys.path.insert(0, "/opt/trn_rl_repo")

import ml_dtypes
import numpy as np

BF16 = ml_dtypes.bfloat16

# ---------------------------------------------------------------------------
# configuration
# ---------------------------------------------------------------------------

FULL_CFG = dict(
    N=100000,      # real nodes
    IN_CH=512,     # input features
    HEADS=8,
    C=16,          # out channels per head
    NC=8,          # cores
    K=14,          # slot-aligned main-pass columns per dst
    R=4,           # blocks per gather group
)

NEG_SLOPE = 0.2
EPS = 1e-16
PAD_ASRC = -1.0e38

PHASE_RANGES = None  # populated by build_program for sim attribution


def _derive(cfg):
    d = dict(cfg)
    d["HC"] = d["HEADS"] * d["C"]                 # 128
    assert d["HC"] == 128
    assert d["IN_CH"] % 128 == 0
    d["KC"] = d["IN_CH"] // 128                   # k-chunks for x@W1
    assert d["N"] % d["NC"] == 0
    d["OWN"] = d["N"] // d["NC"]                  # real nodes per core
    d["BLK"] = (d["OWN"] + 127) // 128            # dst blocks per core
    d["OWN_PAD"] = d["BLK"] * 128
    d["NP"] = d["NC"] * d["OWN_PAD"]              # padded global nodes
    d["PADGID"] = d["OWN"]                        # core-0 pad row (global id)
    d["PADSLOT"] = d["OWN"] % 128                 # its slot in the last block
    d["PADBLK"] = d["OWN"] // 128
    return d


# ---------------------------------------------------------------------------
# host-side prep: edge partitioning / padding / layouts
# ---------------------------------------------------------------------------

def _host_prep(cfg, x, edge_index, W1, att_src1, att_dst1, bias1, W2,
               att_src2, att_dst2, bias2):
    N, NC, OWN, BLK, OWN_PAD, KC, H, C, K = (
        cfg["N"], cfg["NC"], cfg["OWN"], cfg["BLK"], cfg["OWN_PAD"],
        cfg["KC"], cfg["HEADS"], cfg["C"], cfg["K"])
    HC = H * C
    PADGID = cfg["PADGID"]

    src = np.asarray(edge_index[0], dtype=np.int64)
    dst = np.asarray(edge_index[1], dtype=np.int64)

    core = dst // OWN
    ldst = (dst - core * OWN).astype(np.int64)
    srcp = ((src // OWN) * OWN_PAD + (src % OWN)).astype(np.int32)
    blk = ldst // 128
    slot = ldst % 128

    # rank of each edge within its (core, blk, slot) group
    order = np.lexsort((slot, blk, core))
    s_core, s_blk, s_slot = core[order], blk[order], slot[order]
    s_srcp, s_ldst = srcp[order], ldst[order]
    grp = (s_core * BLK + s_blk) * 128 + s_slot
    first = np.ones(len(grp), dtype=bool)
    first[1:] = grp[1:] != grp[:-1]
    starts = np.flatnonzero(first)
    group_start = np.repeat(starts, np.diff(np.append(starts, len(grp))))
    rank = np.arange(len(grp)) - group_start

    # ---- main pass: slot-aligned [128, BLK*K]
    main = rank < K
    srcp_m = np.full((NC, 128, BLK * K), PADGID, dtype=np.int32)
    srcp_m[s_core[main], s_slot[main], s_blk[main] * K + rank[main]] = \
        s_srcp[main]

    # ---- overflow pass: densely packed per block
    ov = ~main
    o_core, o_blk = s_core[ov], s_blk[ov]
    # position among the block's overflow edges
    og = o_core * BLK + o_blk
    ofirst = np.ones(len(og), dtype=bool)
    ofirst[1:] = og[1:] != og[:-1]
    ostarts = np.flatnonzero(ofirst)
    ogroup_start = np.repeat(ostarts, np.diff(np.append(ostarts, len(og))))
    opos = np.arange(len(og)) - ogroup_start

    ocounts = np.zeros((NC, BLK), dtype=np.int64)
    np.add.at(ocounts, (o_core, o_blk), 1)
    Tov = np.ceil(ocounts.max(axis=0) / 128).astype(np.int64)  # may be 0
    offv = np.zeros(BLK, dtype=np.int64)
    offv[1:] = np.cumsum(Tov)[:-1]
    totTov = int(Tov.sum())

    srcp_o = np.full((NC, 128, max(totTov, 1)), PADGID, dtype=np.int32)
    dstl_o = np.zeros((NC, 128, max(totTov, 1)), dtype=np.int32)
    slot_o = np.full((NC, 128, max(totTov, 1)), 999.0, dtype=np.float32)
    pp = opos % 128
    col = offv[o_blk] + opos // 128
    srcp_o[o_core, pp, col] = s_srcp[ov]
    dstl_o[o_core, pp, col] = s_ldst[ov]
    slot_o[o_core, pp, col] = s_slot[ov].astype(np.float32)

    # x in padded per-core layout, pre-transposed for matmul lhsT, bf16,
    # tile-contiguous: xTb[p, b, kc, g] = x[own block b, node g, kc*128 + p]
    xf = np.asarray(x, np.float32)
    in_maps = []
    W1b = np.ascontiguousarray(
        np.asarray(W1, np.float32).reshape(KC, 128, HC)
        .transpose(1, 0, 2).astype(BF16))
    iota = np.ascontiguousarray(np.broadcast_to(
        np.arange(128, dtype=np.float32)[None, :], (128, 128)).astype(BF16))
    attS1 = np.broadcast_to(
        np.asarray(att_src1, np.float32).reshape(1, HC), (128, HC)).copy()
    attD1 = np.broadcast_to(
        np.asarray(att_dst1, np.float32).reshape(1, HC), (128, HC)).copy()
    attS2 = np.broadcast_to(
        np.asarray(att_src2, np.float32).reshape(1, HC), (128, HC)).copy()
    attD2 = np.broadcast_to(
        np.asarray(att_dst2, np.float32).reshape(1, HC), (128, HC)).copy()
    b1b = np.broadcast_to(
        np.asarray(bias1, np.float32).reshape(1, HC), (128, HC)).copy()
    W2b = np.ascontiguousarray(np.asarray(W2, np.float32).astype(BF16))
    b2b = np.broadcast_to(
        np.asarray(bias2, np.float32).reshape(1, C), (128, C)).copy()
    pad8 = np.full((1, 8), PAD_ASRC, np.float32).astype(BF16)
    shared = dict(W1b=W1b, iota=iota, attS1=attS1, attD1=attD1, attS2=attS2,
                  attD2=attD2, b1b=b1b, W2b=W2b, b2b=b2b, pad8=pad8)

    for m in range(NC):
        xs = np.zeros((OWN_PAD, cfg["IN_CH"]), np.float32)
        xs[:OWN] = xf[m * OWN:(m + 1) * OWN]
        t = xs.reshape(BLK, 128, KC, 128).transpose(3, 0, 2, 1)
        im = dict(shared)
        im["xTb"] = np.ascontiguousarray(t.astype(BF16)).reshape(
            128, BLK * KC * 128)
        im["srcp_m"] = np.ascontiguousarray(srcp_m[m])
        im["srcp_o"] = np.ascontiguousarray(srcp_o[m])
        im["dstl_o"] = np.ascontiguousarray(dstl_o[m])
        im["slot_o"] = np.ascontiguousarray(slot_o[m].astype(BF16))
        in_maps.append(im)

    return in_maps, Tov.tolist(), offv.tolist(), totTov


# ---------------------------------------------------------------------------
# device program
# ---------------------------------------------------------------------------

def build_program(cfg, Tov, offv, totTov):
    global PHASE_RANGES
    from concourse import bacc, bass, mybir, tile
    from concourse.masks import make_identity

    f32 = mybir.dt.float32
    bf16 = mybir.dt.bfloat16
    i32 = mybir.dt.int32
    X = mybir.AxisListType.X
    AF = mybir.ActivationFunctionType
    NC, NP, OWN_PAD, BLK, KC, H, C, K, R = (
        cfg["NC"], cfg["NP"], cfg["OWN_PAD"], cfg["BLK"], cfg["KC"],
        cfg["HEADS"], cfg["C"], cfg["K"], cfg["R"])
    HC = H * C
    W = 8 + HC               # table row width: [h (128) | a_src (8)]
    PADSLOT, PADBLK = cfg["PADSLOT"], cfg["PADBLK"]
    nG = (BLK + R - 1) // R  # gather groups

    nc = bacc.Bacc("TRN2", target_bir_lowering=False, debug=False,
                   num_devices=NC)

    # inputs
    t_xTb = nc.dram_tensor("xTb", [128, BLK * KC * 128], bf16,
                           kind="ExternalInput")
    t_W1b = nc.dram_tensor("W1b", [128, KC, HC], bf16, kind="ExternalInput")
    t_iota = nc.dram_tensor("iota", [128, 128], bf16, kind="ExternalInput")
    t_attS1 = nc.dram_tensor("attS1", [128, HC], f32, kind="ExternalInput")
    t_attD1 = nc.dram_tensor("attD1", [128, HC], f32, kind="ExternalInput")
    t_attS2 = nc.dram_tensor("attS2", [128, HC], f32, kind="ExternalInput")
    t_attD2 = nc.dram_tensor("attD2", [128, HC], f32, kind="ExternalInput")
    t_b1b = nc.dram_tensor("b1b", [128, HC], f32, kind="ExternalInput")
    t_W2b = nc.dram_tensor("W2b", [HC, HC], bf16, kind="ExternalInput")
    t_b2b = nc.dram_tensor("b2b", [128, C], f32, kind="ExternalInput")
    t_srcm = nc.dram_tensor("srcp_m", [128, BLK * K], i32,
                            kind="ExternalInput")
    t_srco = nc.dram_tensor("srcp_o", [128, max(totTov, 1)], i32,
                            kind="ExternalInput")
    t_dstlo = nc.dram_tensor("dstl_o", [128, max(totTov, 1)], i32,
                             kind="ExternalInput")
    t_sloto = nc.dram_tensor("slot_o", [128, max(totTov, 1)], bf16,
                             kind="ExternalInput")
    t_pad8 = nc.dram_tensor("pad8", [1, 8], bf16, kind="ExternalInput")
    t_out = nc.dram_tensor("out", [OWN_PAD, C], f32, kind="ExternalOutput")

    # internal DRAM
    table1 = nc.dram_tensor("table1", [NP, W], bf16, kind="Internal",
                            addr_space="Shared")
    table2 = nc.dram_tensor("table2", [NP, W], bf16, kind="Internal",
                            addr_space="Shared")
    t1own = nc.dram_tensor("t1own", [OWN_PAD, W], bf16, kind="Internal")
    h2own = nc.dram_tensor("h2own", [OWN_PAD, W], bf16, kind="Internal")
    adst1 = nc.dram_tensor("adst1", [OWN_PAD, H], bf16, kind="Internal")
    adst2 = nc.dram_tensor("adst2", [OWN_PAD, H], bf16, kind="Internal")

    def fv(ap, dims, extra_offset=0):
        """View `ap` with custom free-dim [step, count] pairs."""
        return bass.AP(ap.tensor, ap.offset + extra_offset, [ap.ap[0]] + dims)

    marks = [("init", 0)]

    def mark(name):
        marks.append((name, sum(len(b.instructions)
                                for b in nc.m.functions[0].blocks)))

    with tile.TileContext(nc) as tc:
        with (
            tc.tile_pool(name="const", bufs=1) as cpool,
        ):
            # ---------------- constants ----------------
            W1_sb = cpool.tile([128, KC * HC], bf16, tag="w1")
            nc.sync.dma_start(
                out=fv(W1_sb[:], [[HC, KC], [1, HC]]), in_=t_W1b[:, :, :])
            iota_sb = cpool.tile([128, 128], bf16, tag="iota")
            nc.sync.dma_start(out=iota_sb[:], in_=t_iota[:, :])
            attS1_sb = cpool.tile([128, HC], f32, tag="attS1")
            nc.sync.dma_start(out=attS1_sb[:], in_=t_attS1[:, :])
            attD1_sb = cpool.tile([128, HC], f32, tag="attD1")
            nc.sync.dma_start(out=attD1_sb[:], in_=t_attD1[:, :])
            attS2_sb = cpool.tile([128, HC], f32, tag="attS2")
            nc.sync.dma_start(out=attS2_sb[:], in_=t_attS2[:, :])
            attD2_sb = cpool.tile([128, HC], f32, tag="attD2")
            nc.sync.dma_start(out=attD2_sb[:], in_=t_attD2[:, :])
            b1_sb = cpool.tile([128, HC], f32, tag="b1")
            nc.sync.dma_start(out=b1_sb[:], in_=t_b1b[:, :])
            b1bf_sb = cpool.tile([128, HC], bf16, tag="b1bf")
            nc.scalar.copy(out=b1bf_sb[:], in_=b1_sb[:])
            W2_sb = cpool.tile([HC, HC], bf16, tag="w2")
            nc.sync.dma_start(out=W2_sb[:], in_=t_W2b[:, :])
            b2_sb = cpool.tile([128, C], f32, tag="b2")
            nc.sync.dma_start(out=b2_sb[:], in_=t_b2b[:, :])
            ident = cpool.tile([128, 128], bf16, tag="ident")
            make_identity(nc, ident[:])

            # edge index tables, SBUF-resident for both layers
            srcm_sb = cpool.tile([128, BLK * K], i32, tag="srcm")
            nc.sync.dma_start(out=srcm_sb[:], in_=t_srcm[:, :])
            srco_sb = cpool.tile([128, max(totTov, 1)], i32, tag="srco")
            nc.sync.dma_start(out=srco_sb[:], in_=t_srco[:, :])
            dstlo_sb = cpool.tile([128, max(totTov, 1)], i32, tag="dstlo")
            nc.sync.dma_start(out=dstlo_sb[:], in_=t_dstlo[:, :])
            sloto_sb = cpool.tile([128, max(totTov, 1)], bf16, tag="sloto")
            nc.sync.dma_start(out=sloto_sb[:], in_=t_sloto[:, :])

            # finish2 accumulators (persist across blocks)
            red_all = cpool.tile([128, BLK * C], f32, tag="redall")
            sm_all = cpool.tile([128, BLK], f32, tag="small")

            attS1bf = cpool.tile([128, HC], bf16, tag="attS1bf")
            nc.scalar.copy(out=attS1bf[:], in_=attS1_sb[:])
            attD1bf = cpool.tile([128, HC], bf16, tag="attD1bf")
            nc.scalar.copy(out=attD1bf[:], in_=attD1_sb[:])
            attS2bf = cpool.tile([128, HC], bf16, tag="attS2bf")
            nc.scalar.copy(out=attS2bf[:], in_=attS2_sb[:])
            attD2bf = cpool.tile([128, HC], bf16, tag="attD2bf")
            nc.scalar.copy(out=attD2bf[:], in_=attD2_sb[:])

            def group_scores(pool, hb, adt, nB, b0, attS, attD):
                """Batched a_src/a_dst scores for a group's h tile.

                hb: [128, nB*W] bf16, h at cols i*W..i*W+HC; scores are
                written into cols i*W+HC..(i+1)*W. adt: [128, nB*H] bf16.
                """
                t1g = pool.tile([128, R * HC], f32, tag="t1g")
                nc.vector.tensor_mul(
                    out=fv(t1g[:], [[HC, nB], [1, HC]]),
                    in0=fv(hb[:], [[W, nB], [1, HC]]),
                    in1=fv(attS[:], [[0, nB], [1, HC]]))
                asg = pool.tile([128, R * H], f32, tag="asg")
                nc.vector.reduce_sum(
                    out=fv(asg[:], [[H, nB], [1, H]]),
                    in_=fv(t1g[:], [[HC, nB], [C, H], [1, C]]), axis=X)
                t2g = pool.tile([128, R * HC], f32, tag="t2g")
                nc.vector.tensor_mul(
                    out=fv(t2g[:], [[HC, nB], [1, HC]]),
                    in0=fv(hb[:], [[W, nB], [1, HC]]),
                    in1=fv(attD[:], [[0, nB], [1, HC]]))
                adg = pool.tile([128, R * H], f32, tag="adg")
                nc.vector.reduce_sum(
                    out=fv(adg[:], [[H, nB], [1, H]]),
                    in_=fv(t2g[:], [[HC, nB], [C, H], [1, C]]), axis=X)
                nc.scalar.copy(
                    out=fv(hb[:], [[W, nB], [1, H]], extra_offset=HC),
                    in_=asg[:, :nB * H])
                nc.scalar.copy(out=adt[:, :nB * H], in_=adg[:, :nB * H])

            mark("phaseA")
            # ---------------- phase A: own-node table1 ----------------
            # grouped by R blocks to amortize DMA dispatch overhead
            with (
                tc.tile_pool(name="pa", bufs=3) as pa,
                tc.tile_pool(name="pa_ps", bufs=3, space="PSUM") as pa_ps,
            ):
                for g in range(nG):
                    b0 = g * R
                    nB = min(R, BLK - b0)
                    xt = pa.tile([128, nB * KC * 128], bf16, tag="xt")
                    nc.sync.dma_start(
                        out=xt[:],
                        in_=t_xTb[:, b0 * KC * 128:(b0 + nB) * KC * 128])
                    hb = pa.tile([128, nB * W], bf16, tag="hb")
                    adt = pa.tile([128, nB * H], bf16, tag="adt")
                    for i in range(nB):
                        ph = pa_ps.tile([128, HC], f32, tag="ph")
                        for k in range(KC):
                            nc.tensor.matmul(
                                out=ph[:],
                                lhsT=xt[:, (i * KC + k) * 128:
                                        (i * KC + k + 1) * 128],
                                rhs=W1_sb[:, k * HC:(k + 1) * HC],
                                start=(k == 0), stop=(k == KC - 1))
                        nc.scalar.copy(
                            out=hb[:, i * W:i * W + HC], in_=ph[:])
                    group_scores(pa, hb, adt, nB, b0, attS1bf, attD1bf)
                    nc.sync.dma_start(
                        out=fv(t1own[b0 * 128:(b0 + 1) * 128, :],
                               [[128 * W, nB], [1, W]]),
                        in_=hb[:])
                    nc.sync.dma_start(
                        out=fv(adst1[b0 * 128:(b0 + 1) * 128, :],
                               [[128 * H, nB], [1, H]]),
                        in_=adt[:])

            # mark the PAD row: its a_src becomes -1e38 so ex == 0
            nc.sync.dma_start(
                out=t1own[cfg["PADGID"]:cfg["PADGID"] + 1, HC:W],
                in_=t_pad8[:, :])

            # prefetch edge1's a_dst data so it overlaps the AllGather
            ad1_all = cpool.tile([128, BLK * H], bf16, tag="ad1all")
            nc.sync.dma_start(
                out=ad1_all[:],
                in_=fv(adst1[0:128, :], [[128 * H, BLK], [1, H]]))
            if totTov:
                gdo1_all = cpool.tile([128, totTov * H], bf16, tag="gdo1all")
                nc.gpsimd.indirect_dma_start(
                    out=gdo1_all[:], out_offset=None,
                    in_=adst1[:, :],
                    in_offset=bass.IndirectOffsetOnAxis(
                        ap=dstlo_sb[:, :totTov], axis=0))
            else:
                gdo1_all = None

            mark("allgather1")
            nc.gpsimd.collective_compute(
                "AllGather",
                mybir.AluOpType.bypass,
                replica_groups=[list(range(NC))],
                ins=[t1own[:].opt()],
                outs=[table1[:].opt()],
            )

            # ---------------- edge phase (shared between layers) ----------
            def edge_phase(tag, tab, ad_all, gdo_all, fblock, fgroup):
                with (
                    tc.tile_pool(name=f"eg{tag}", bufs=4) as eg,
                    tc.tile_pool(name=f"ef{tag}", bufs=3) as ef,
                    tc.tile_pool(name=f"ff{tag}", bufs=4) as ff,
                    tc.tile_pool(name=f"eps{tag}", bufs=3, space="PSUM") as eps,
                    tc.tile_pool(name=f"fps{tag}", bufs=2, space="PSUM") as fps,
                ):
                    for g in range(nG):
                        b0 = g * R
                        nB = min(R, BLK - b0)
                        tov = [Tov[b0 + i] for i in range(nB)]
                        gTov = sum(tov)
                        o0 = offv[b0]

                        # batched main gather: [128, nB*K rows of W]
                        gm = eg.tile([128, nB * K * W], bf16, tag="gm")
                        nc.gpsimd.indirect_dma_start(
                            out=gm[:], out_offset=None,
                            in_=tab[:, :],
                            in_offset=bass.IndirectOffsetOnAxis(
                                ap=srcm_sb[:, b0 * K:(b0 + nB) * K], axis=0))
                        if gTov:
                            go = eg.tile([128, gTov * W], bf16, tag="go")
                            nc.gpsimd.indirect_dma_start(
                                out=go[:], out_offset=None,
                                in_=tab[:, :],
                                in_offset=bass.IndirectOffsetOnAxis(
                                    ap=srco_sb[:, o0:o0 + gTov], axis=0))
                            # one-hot for the group's overflow edges
                            Pm = ef.tile([128, gTov * 128], bf16, tag="Pm")
                            nc.vector.tensor_tensor(
                                out=fv(Pm[:], [[128, gTov], [1, 128]]),
                                in0=fv(sloto_sb[:, o0:o0 + gTov],
                                       [[1, gTov], [0, 128]]),
                                in1=fv(iota_sb[:], [[0, gTov], [1, 128]]),
                                op=mybir.AluOpType.is_equal)
                            # overflow alpha -> ex (whole group at once)
                            axo = ef.tile([128, gTov * H], bf16, tag="axo")
                            nc.vector.tensor_add(
                                out=axo[:],
                                in0=fv(go[:], [[W, gTov], [1, H]],
                                       extra_offset=HC),
                                in1=gdo_all[:, o0 * H:(o0 + gTov) * H])
                            nc.vector.scalar_tensor_tensor(
                                out=axo[:], in0=axo[:], scalar=NEG_SLOPE,
                                in1=axo[:], op0=mybir.AluOpType.mult,
                                op1=mybir.AluOpType.max)
                            exo = ef.tile([128, gTov * H], bf16, tag="exo")
                            nc.scalar.activation(out=exo[:], in_=axo[:],
                                                 func=AF.Exp)
                            # weight features in place + append ex
                            nc.vector.tensor_mul(
                                out=fv(go[:], [[W, gTov], [C, H], [1, C]]),
                                in0=fv(go[:], [[W, gTov], [C, H], [1, C]]),
                                in1=fv(exo[:], [[H, gTov], [1, H], [0, C]]))
                            nc.scalar.copy(
                                out=fv(go[:], [[W, gTov], [1, H]],
                                       extra_offset=HC),
                                in_=exo[:])

                        # main alpha -> ex (whole group at once)
                        axm = ef.tile([128, nB * K * H], bf16, tag="axm")
                        nc.vector.tensor_add(
                            out=axm[:],
                            in0=fv(gm[:], [[K * W, nB], [W, K], [1, H]],
                                   extra_offset=HC),
                            in1=fv(ad_all[:, b0 * H:(b0 + nB) * H],
                                   [[H, nB], [0, K], [1, H]]))
                        nc.vector.scalar_tensor_tensor(
                            out=axm[:], in0=axm[:], scalar=NEG_SLOPE,
                            in1=axm[:], op0=mybir.AluOpType.mult,
                            op1=mybir.AluOpType.max)
                        exm = ef.tile([128, nB * K * H], bf16, tag="exm")
                        nc.scalar.activation(out=exm[:], in_=axm[:],
                                             func=AF.Exp)
                        # expand ex over channels on the Activation engine so
                        # the big weighting multiply runs in DVE 2x mode
                        exe = ef.tile([128, nB * K * HC], bf16, tag="exe")
                        nc.scalar.copy(
                            out=exe[:],
                            in_=fv(exm[:], [[H, nB * K], [1, H], [0, C]]))
                        nc.vector.tensor_mul(
                            out=fv(gm[:], [[W, nB * K], [1, HC]]),
                            in0=fv(gm[:], [[W, nB * K], [1, HC]]),
                            in1=exe[:])
                        nc.scalar.copy(
                            out=fv(gm[:], [[W, nB * K], [1, H]],
                                   extra_offset=HC),
                            in_=exm[:])

                        oo = 0
                        gst = {}
                        for i in range(nB):
                            b = b0 + i
                            nd = eps.tile([128, W], f32, tag="nd")
                            nmm = K + tov[i]
                            for j in range(K):
                                nc.tensor.matmul(
                                    out=nd[:],
                                    lhsT=ident[:],
                                    rhs=gm[:, (i * K + j) * W:
                                           (i * K + j + 1) * W],
                                    start=(j == 0), stop=(j == nmm - 1))
                            for j in range(tov[i]):
                                nc.tensor.matmul(
                                    out=nd[:],
                                    lhsT=Pm[:, (oo + j) * 128:
                                            (oo + j + 1) * 128],
                                    rhs=go[:, (oo + j) * W:(oo + j + 1) * W],
                                    start=False, stop=(K + j == nmm - 1))
                            oo += tov[i]
                            fblock(i, b, nd, ef, ff, fps, gst)
                        fgroup(b0, nB, ef, ff, gst)

            # ---------------- layer-1 finisher ----------------------------
            # x2 = elu(num/den + b1); h2 = x2 @ W2; table row = [h2 | a_src2]
            def fblock1(i, b, nd, ef, ff, fps, gst):
                if i == 0:
                    hb2g = ef.tile([128, R * W], bf16, tag="hb2g")
                    gst["hb2"] = hb2g
                dr = ff.tile([128, H], f32, tag="dr")
                nc.vector.tensor_scalar_add(dr[:], nd[:, HC:W], EPS)
                nc.vector.reciprocal(out=dr[:], in_=dr[:])
                g = ff.tile([128, HC], bf16, tag="g")
                nc.vector.tensor_tensor(
                    out=fv(g[:], [[C, H], [1, C]]),
                    in0=fv(nd[:], [[C, H], [1, C]]),
                    in1=fv(dr[:], [[1, H], [0, C]]),
                    op=mybir.AluOpType.mult)
                nc.vector.tensor_add(out=g[:], in0=g[:], in1=b1bf_sb[:])
                # ELU: x2 = max(g,0) - 1 + exp(min(g,0))
                tn = ff.tile([128, HC], bf16, tag="tn")
                nc.vector.tensor_scalar_min(tn[:], g[:], 0.0)
                te = ff.tile([128, HC], bf16, tag="te")
                nc.scalar.activation(out=te[:], in_=tn[:], func=AF.Exp)
                nc.vector.tensor_scalar(
                    out=g[:], in0=g[:], scalar1=0.0, scalar2=-1.0,
                    op0=mybir.AluOpType.max, op1=mybir.AluOpType.add)
                x2 = ff.tile([128, HC], bf16, tag="x2")
                nc.vector.tensor_add(out=x2[:], in0=g[:], in1=te[:])
                # h2 = x2 @ W2 via PE transpose then matmul
                xtp = fps.tile([128, 128], bf16, tag="xtp")
                nc.tensor.transpose(out=xtp[:], in_=x2[:], identity=ident[:])
                xts = ff.tile([128, 128], bf16, tag="xts")
                nc.scalar.copy(out=xts[:], in_=xtp[:])
                h2p = fps.tile([128, HC], f32, tag="h2p")
                nc.tensor.matmul(out=h2p[:], lhsT=xts[:], rhs=W2_sb[:],
                                 start=True, stop=True)
                nc.scalar.copy(out=gst["hb2"][:, i * W:i * W + HC],
                               in_=h2p[:])

            def fgroup1(b0, nB, ef, ff, gst):
                hb2 = gst["hb2"]
                ad2 = ef.tile([128, R * H], bf16, tag="ad2g")
                group_scores(ef, hb2, ad2, nB, b0, attS2bf, attD2bf)
                nc.sync.dma_start(
                    out=fv(h2own[b0 * 128:(b0 + 1) * 128, :],
                           [[128 * W, nB], [1, W]]),
                    in_=hb2[:, :nB * W])
                nc.sync.dma_start(
                    out=fv(adst2[b0 * 128:(b0 + 1) * 128, :],
                           [[128 * H, nB], [1, H]]),
                    in_=ad2[:, :nB * H])

            # ---------------- layer-2 finisher ----------------------------
            # out = log_softmax(mean_h(num/den) + b2); Ln batched at the end
            def fblock2(i, b, nd, ef, ff, fps, gst):
                if i == 0:
                    ggg = ef.tile([128, R * HC], f32, tag="ggg")
                    gst["gg"] = ggg
                dr = ff.tile([128, H], f32, tag="dr")
                nc.vector.tensor_scalar_add(dr[:], nd[:, HC:W], EPS)
                nc.vector.reciprocal(out=dr[:], in_=dr[:])
                nc.vector.tensor_tensor(
                    out=fv(gst["gg"][:, i * HC:(i + 1) * HC],
                           [[C, H], [1, C]]),
                    in0=fv(nd[:], [[C, H], [1, C]]),
                    in1=fv(dr[:], [[1, H], [0, C]]),
                    op=mybir.AluOpType.mult)

            def fgroup2(b0, nB, ef, ff, gst):
                gg = gst["gg"]
                red = red_all[:, b0 * C:(b0 + nB) * C]
                nc.vector.reduce_sum(
                    out=fv(red, [[C, nB], [1, C]]),
                    in_=fv(gg[:], [[HC, nB], [1, C], [C, H]]), axis=X)
                nc.vector.tensor_scalar(
                    out=red, in0=red, scalar1=1.0 / H, scalar2=0.0,
                    op0=mybir.AluOpType.mult, op1=mybir.AluOpType.add)
                nc.vector.tensor_tensor(
                    out=fv(red, [[C, nB], [1, C]]),
                    in0=fv(red, [[C, nB], [1, C]]),
                    in1=fv(b2_sb[:], [[0, nB], [1, C]]),
                    op=mybir.AluOpType.add)
                mxg = ff.tile([128, R], f32, tag="mxg")
                nc.vector.reduce_max(
                    out=mxg[:, :nB], in_=fv(red, [[C, nB], [1, C]]), axis=X)
                nc.vector.tensor_tensor(
                    out=fv(red, [[C, nB], [1, C]]),
                    in0=fv(red, [[C, nB], [1, C]]),
                    in1=fv(mxg[:, :nB], [[1, nB], [0, C]]),
                    op=mybir.AluOpType.subtract)
                esg = ff.tile([128, R * C], f32, tag="esg")
                nc.scalar.activation(out=esg[:, :nB * C], in_=red,
                                     func=AF.Exp)
                nc.vector.reduce_sum(
                    out=sm_all[:, b0:b0 + nB],
                    in_=fv(esg[:], [[C, nB], [1, C]]), axis=X)

            # ---------------- run both layers ------------------------------
            mark("edge1")
            edge_phase("1", table1, ad1_all, gdo1_all, fblock1, fgroup1)

            nc.sync.dma_start(
                out=h2own[cfg["PADGID"]:cfg["PADGID"] + 1, HC:W],
                in_=t_pad8[:, :])

            # prefetch edge2's a_dst data so it overlaps the AllGather
            ad2_all = cpool.tile([128, BLK * H], bf16, tag="ad2all")
            nc.sync.dma_start(
                out=ad2_all[:],
                in_=fv(adst2[0:128, :], [[128 * H, BLK], [1, H]]))
            if totTov:
                gdo2_all = cpool.tile([128, totTov * H], bf16, tag="gdo2all")
                nc.gpsimd.indirect_dma_start(
                    out=gdo2_all[:], out_offset=None,
                    in_=adst2[:, :],
                    in_offset=bass.IndirectOffsetOnAxis(
                        ap=dstlo_sb[:, :totTov], axis=0))
            else:
                gdo2_all = None

            mark("allgather2")
            nc.gpsimd.collective_compute(
                "AllGather",
                mybir.AluOpType.bypass,
                replica_groups=[list(range(NC))],
                ins=[h2own[:].opt()],
                outs=[table2[:].opt()],
            )

            mark("edge2")
            edge_phase("2", table2, ad2_all, gdo2_all, fblock2, fgroup2)

            mark("finish")
            # batched log-softmax denominator + single output write
            with tc.tile_pool(name="fin", bufs=1) as fin:
                ls = fin.tile([128, BLK], f32, tag="ls")
                nc.scalar.activation(out=ls[:], in_=sm_all[:], func=AF.Ln)
                nc.vector.tensor_tensor(
                    out=fv(red_all[:], [[C, BLK], [1, C]]),
                    in0=fv(red_all[:], [[C, BLK], [1, C]]),
                    in1=fv(ls[:], [[1, BLK], [0, C]]),
                    op=mybir.AluOpType.subtract)
                nc.sync.dma_start(
                    out=fv(t_out[0:128, :], [[128 * C, BLK], [1, C]]),
                    in_=red_all[:])

    mark("end")
    nc.compile()
    PHASE_RANGES = [
        (marks[i][0], marks[i][1], marks[i + 1][1])
        for i in range(len(marks) - 1)
    ]
    return nc


# ---------------------------------------------------------------------------
# entry point
# ---------------------------------------------------------------------------

def _run(cfg, inputs, trace=False):
    from concourse.bass_utils import run_bass_kernel_spmd

    cfg = _derive(cfg)
    in_maps, Tov, offv, totTov = _host_prep(cfg, **inputs)
    nc = build_program(cfg, Tov, offv, totTov)
    res = run_bass_kernel_spmd(
        nc, in_maps, core_ids=list(range(cfg["NC"])), trace=trace)
    outs = []
    for m in range(cfg["NC"]):
        outs.append(res.results[m]["out"][:cfg["OWN"]])
    full = np.concatenate(outs, axis=0)
    return full, res


def kernel(x, edge_index, W1, att_src1, att_dst1, bias1, W2, att_src2,
           att_dst2, bias2):
    inputs = dict(x=np.asarray(x, np.float32),
                  edge_index=np.asarray(edge_index),
                  W1=W1, att_src1=att_src1, att_dst1=att_dst1, bias1=bias1,
                  W2=W2, att_src2=att_src2, att_dst2=att_dst2, bias2=bias2)
    out, _ = _run(FULL_CFG, inputs, trace=False)
    return out
